# revision 29
# baseline (speedup 1.0000x reference)
"""Bass/Trainium2 kernel for nn_ExpertMLP (soft-blended 8-expert MLP with
BatchNorm between the two layers).

Math (per sample b):
    h  = sum_e coef[b,e] * (x[b] @ w1[e])  + coef[b] @ b1
    hn = (h - mean_B(h)) * rsqrt(var_B(h) + eps) * gamma + beta
    h1 = elu(hn)
    out= sum_e coef[b,e] * (h1[b] @ w2[e]) + coef[b] @ b2

Sharding: HID (512) is split 64-per-core across 8 cores. Each core processes
the FULL batch for its HID slice, so the BatchNorm batch statistics are fully
local (no collective), and the per-expert weights are sharded (not
replicated). Layer 2 contracts only the local HID slice, so each core emits a
partial output [OUT, B]; the host sums the 8 partials and transposes.

Fast path (b1/b2 zero, gamma ones, beta zero — true for this problem's
setup_inputs): bf16 matmul inputs, host-precomputed coefficient broadcast
tiles, rsqrt via exp(-0.5*ln(var+eps)) so every ACT func (Ln/Exp/Relu) lives
in one activation-table set (no 1.3us table reloads), ELU's "-1" fused into
the per-pair coefficient multiplies, one consolidated DMA per tensor (each
HWDGE DMA occupies its ring ~2.3us on HW), and a staggered-reset hardware
loop (For_i(staggered_reset=True)) so consecutive iterations overlap instead
of paying a full all-engine drain at every back-edge. General path (nonzero
biases / affine) keeps the bias matmuls and BN affine.

On-chip layout is transposed (features on partitions, batch on the free dim):
  - L1: yp_pair[(e0|e1)*64hid, b] = sum_k W1L[p,k].T @ xT[k, b]
  - blend: multiply by per-expert coefficient tiles (host-broadcast), then a
    matmul with a tiled identity [I;I|I;I] sums the two expert halves of each
    pair, accumulating over the 4 pairs in PSUM, and duplicates h to both
    partition halves.
  - BN: bn_stats/bn_aggr over the free (batch) dim; rstd = exp(-0.5 ln(var+eps)).
  - ELU: t = min(exp(hn),1) + relu(hn); u_p = (t - 1) * coef_p  (fused).
  - L2: out[m] += W2L[p,m].T @ u_p  (m-major; each PSUM bank is copied to
    a packed SBUF tile and stored 2 m-tiles per DMA).
"""

import sys

sys.path.insert(0, "/opt/trn_rl_repo")

import numpy as np

E, IN, HID, OUT, B = 8, 512, 512, 512, 1024
BN_EPS = 1e-5
N_CORES = 8
HSL = HID // N_CORES  # 64: per-core hid slice
NPAIR = E // 2  # 4 expert pairs
KT1 = IN // 128  # 4 contraction tiles for layer 1
MT2 = OUT // 128  # 4 output row-tiles for layer 2
NBH = B // 512  # 2 batch halves (PSUM free-dim limit)
CBB_W = B + 128 + OUT  # packed coefT | b1dup | b2 widths (general path)

_CACHED_FAST = None
_CACHED_GEN = None


def build_nc(n_reps: int = 1, trace_sim: bool = False, loop_iters: int = 0,
             general: bool = False):
    from contextlib import ExitStack, nullcontext

    import concourse.bass as bass
    import concourse.tile as tile
    from concourse import bacc, mybir

    f32 = mybir.dt.float32
    bf16 = mybir.dt.bfloat16
    Alu = mybir.AluOpType
    Act = mybir.ActivationFunctionType

    nc = bacc.Bacc(
        "TRN2", target_bir_lowering=False, debug=False, num_devices=N_CORES
    )

    # one DMA per tensor: each HWDGE DMA occupies its ring ~2.3us on HW
    # (issue + transfer + completion receipt, unpipelined), so batch hard.
    xT = nc.dram_tensor("xT", [128, KT1, B], bf16, kind="ExternalInput")
    w1l = nc.dram_tensor("w1l", [128, NPAIR * KT1 * 128 + 128], bf16,
                         kind="ExternalInput")  # w1 tiles + idd appended
    w2l = nc.dram_tensor("w2l", [128, NPAIR * MT2 * 128], bf16, kind="ExternalInput")
    cb4 = nc.dram_tensor("cb4", [128, NPAIR, B], bf16, kind="ExternalInput")
    if general:
        cbb = nc.dram_tensor("cbb", [E, CBB_W], bf16, kind="ExternalInput")
        gb = nc.dram_tensor("gb", [128, 2], f32, kind="ExternalInput")
    outT = nc.dram_tensor("outT", [128, MT2, NBH, 512], bf16,
                          kind="ExternalOutput")

    with ExitStack() as ctx:
        tc = ctx.enter_context(tile.TileContext(nc, trace_sim=trace_sim))
        ins = ctx.enter_context(tc.tile_pool(name="ins", bufs=2))
        blend = ctx.enter_context(tc.tile_pool(name="blend", bufs=6))
        mids = ctx.enter_context(tc.tile_pool(name="mids", bufs=4))
        small = ctx.enter_context(tc.tile_pool(name="small", bufs=2))
        ups = ctx.enter_context(tc.tile_pool(name="ups", bufs=8))
        outs = ctx.enter_context(tc.tile_pool(name="outs", bufs=6))
        yps = ctx.enter_context(tc.tile_pool(name="yps", bufs=3, space="PSUM"))
        hps = ctx.enter_context(tc.tile_pool(name="hps", bufs=2, space="PSUM"))
        ops = ctx.enter_context(tc.tile_pool(name="ops", bufs=3, space="PSUM"))

        loop_cm = (tc.For_i(0, loop_iters, 1, staggered_reset=True)
                   if loop_iters else nullcontext())
        with loop_cm:
          for _rep in range(n_reps):
            # ---- input loads.  SP: x + w1 + idd (earliest-needed);
            # gpsimd SWDGE: cb4 + w2 (needed a bit later); ACT kept free
            # for compute + the half-0 output stores.
            xall = ins.tile([128, KT1, B], bf16, tag="xts", name="xts", bufs=2)
            cbt = ins.tile([128, NPAIR, B], bf16, tag="cbt", name="cbt", bufs=2)
            if general:
                cbbt = ins.tile([E, CBB_W], bf16, tag="cbbt")
                nc.sync.dma_start(cbbt[:], cbb[:])
            # lead-in: SP carries x[k=0,1], gpsimd(SWDGE) x[k=2,3], ACT the
            # fused w1+idd block; cbt follows on ACT, w2 on SP (needed last)
            nc.sync.dma_start(xall[:, 0:2, :], xT[:, 0:2, :])
            w1i = ins.tile([128, NPAIR * KT1 * 128 + 128], bf16, tag="w1t",
                           name="w1t", bufs=2)
            nc.scalar.dma_start(w1i[:], w1l[:])
            nc.gpsimd.dma_start(xall[:, 2:4, :], xT[:, 2:4, :])
            iddt = w1i[:, NPAIR * KT1 * 128 : NPAIR * KT1 * 128 + 128]
            nc.scalar.dma_start(cbt[:], cb4[:])
            w2all = ins.tile([128, NPAIR, MT2, 128], bf16, tag="w2t",
                             name="w2t", bufs=2)
            w2v = w2all.rearrange("i p m j -> i (p m j)")
            nc.sync.dma_start(w2v[:], w2l[:])
            if general:
                gbt = small.tile([128, 2], f32, tag="gbt")
                nc.sync.dma_start(gbt[:], gb[:])
                ct = cbbt[:, 0:B]
                b1t = cbbt[:, B : B + 128]
                b2t = cbbt[:, B + 128 : CBB_W]
                gm = gbt[:, 0:1]
                bt = gbt[:, 1:2]

            # ---- layer 1 + blend + pair-sum ---------------------------------
            # All 8 yp matmul groups run back-to-back; each pair-sum
            # id-matmul is slotted in only after its blend (DVE) has had a
            # full matmul-group's time to finish, so the ~0.6us cross-engine
            # hop latency never stalls the PE.
            stats = small.tile([128, NBH, 6], f32, tag="stats")
            h_ps = [hps.tile([128, 512], f32, tag="hps", name="hp")
                    for _ in range(NBH)]
            if general:
                for bh in range(NBH):
                    nc.tensor.matmul(
                        h_ps[bh][:], b1t, ct[:, bh * 512 : bh * 512 + 512],
                        start=True, stop=False,
                    )
            bls = {}
            for bh in range(NBH):
                bsl = slice(bh * 512, (bh + 1) * 512)
                for p in range(NPAIR):
                    yp = yps.tile([128, 512], f32, tag="yps", name="yp")
                    for k in range(KT1):
                        nc.tensor.matmul(
                            yp[:],
                            w1i[:, (p * KT1 + k) * 128 : (p * KT1 + k + 1) * 128],
                            xall[:, k, bsl],
                            start=(k == 0), stop=(k == KT1 - 1),
                        )
                    bl = blend.tile([128, 512], bf16, tag="bl", name="bl")
                    nc.vector.tensor_mul(bl[:], yp[:], cbt[:, p, bsl])
                    bls[(bh, p)] = bl
                    if bh == 1:
                        # half-0's pair-sum rides between half-1's yp groups
                        nc.tensor.matmul(
                            h_ps[0][:], iddt, bls[(0, p)][:],
                            start=(p == 0 and not general),
                            stop=(p == NPAIR - 1),
                        )
            nc.vector.bn_stats(out=stats[:, 0, :], in_=h_ps[0][:])
            for p in range(NPAIR):
                nc.tensor.matmul(
                    h_ps[1][:], iddt, bls[(1, p)][:],
                    start=(p == 0 and not general), stop=(p == NPAIR - 1),
                )
            nc.vector.bn_stats(out=stats[:, 1, :], in_=h_ps[1][:])

            # ---- batch-norm: rstd = exp(-0.5 ln(var + eps)) -----------------
            # (Ln/Exp/Relu/Copy share one ACT table set -> no 1.3us reloads)
            eps = small.tile([128, 1], f32, tag="eps")
            nc.vector.memset(eps, BN_EPS)
            zzb = small.tile([128, 1], f32, tag="zzb")
            nc.vector.memset(zzb, 0.0)
            mv = small.tile([128, 2], f32, tag="mv")
            nc.vector.bn_aggr(out=mv[:], in_=stats[:])
            lv = small.tile([128, 1], f32, tag="lv")
            nc.scalar.activation(lv[:], mv[:, 1:2], Act.Ln, bias=eps[:])
            if general:
                rstd = small.tile([128, 1], f32, tag="rstd")
                nc.scalar.activation(rstd[:], lv[:], Act.Exp, bias=zzb[:], scale=-0.5)
                ns = small.tile([128, 1], f32, tag="ns")
                nc.vector.tensor_mul(ns[:], rstd[:], gm)
                nb0 = small.tile([128, 1], f32, tag="nb0")
                nc.vector.tensor_mul(nb0[:], mv[:, 0:1], ns[:])
                nb = small.tile([128, 1], f32, tag="nb")
                nc.vector.tensor_sub(nb[:], bt, nb0[:])
            else:
                ns = small.tile([128, 1], f32, tag="ns")
                nc.scalar.activation(ns[:], lv[:], Act.Exp, bias=zzb[:], scale=-0.5)
                nb = small.tile([128, 1], f32, tag="nb")
                nc.vector.scalar_tensor_tensor(
                    out=nb[:], in0=mv[:, 0:1], scalar=-1.0, in1=ns[:],
                    op0=Alu.mult, op1=Alu.mult,
                )
            # keep the PE activity window warm through the BN tail
            for _w in range(2):
                wps = ops.tile([128, 512], f32, tag="ops", name="wps")
                nc.tensor.matmul(
                    wps[:], iddt, xall[:, _w, 0:512], start=True, stop=True
                )

            # ---- ELU + blend + layer 2 --------------------------------------
            # hn is computed on DVE (the only engine that touches the hp PSUM
            # banks - Tile serializes cross-engine same-bank PSUM access, so
            # ACT reads hn from SBUF instead). t1 = elu(hn) pieces use
            # fast-mode tensor_scalar/tensor_tensor ops.
            zz = small.tile([128, 1], f32, tag="zz")
            nc.vector.memset(zz, 0.0)
            for bh in range(NBH):
                bsl = slice(bh * 512, (bh + 1) * 512)
                hn = mids.tile([128, 512], bf16, tag="hn", name="hn")
                nc.vector.tensor_scalar(
                    out=hn[:], in0=h_ps[bh][:], scalar1=ns[:], scalar2=nb[:],
                    op0=Alu.mult, op1=Alu.add,
                )
                rl = mids.tile([128, 512], bf16, tag="rl", name="rl")
                nc.vector.tensor_scalar(
                    out=rl[:], in0=hn[:], scalar1=0.0, scalar2=None,
                    op0=Alu.max,
                )
                expd = mids.tile([128, 512], bf16, tag="expd", name="expd")
                nc.scalar.activation(expd[:], hn[:], Act.Exp, bias=zz[:])
                a = mids.tile([128, 512], bf16, tag="a", name="a")
                nc.vector.tensor_scalar(
                    out=a[:], in0=expd[:], scalar1=1.0, scalar2=1.0,
                    op0=Alu.min, op1=Alu.subtract,
                )
                t1 = mids.tile([128, 512], bf16, tag="t1", name="t1")
                nc.vector.tensor_add(t1[:], a[:], rl[:])
                if bh == 0:
                    # mid-gap PE keep-warm riding on the ELU intermediates
                    for wi, wsrc in enumerate((hn, expd, a)):
                        wps2 = ops.tile([128, 512], f32, tag="ops",
                                        name="wps")
                        nc.tensor.matmul(
                            wps2[:], iddt, wsrc[:], start=True, stop=True
                        )
                us = []
                for p in range(NPAIR):
                    u = ups.tile([128, 512], bf16, tag="u", name="u")
                    nc.vector.tensor_mul(u[:], t1[:], cbt[:, p, bsl])
                    us.append(u)
                oph = [ops.tile([128, 512], f32, tag="ops", name="op")
                       for _m in range(MT2)]
                if general:
                    for m in range(MT2):
                        nc.tensor.matmul(
                            oph[m][:], b2t[:, m * 128 : (m + 1) * 128],
                            ct[:, bsl], start=True, stop=False,
                        )
                otb = outs.tile([128, MT2, 512], bf16, tag="otb", name="otb")
                # m-major: each op bank completes in turn; copy it into the
                # packed out tile, store 2 m-tiles per DMA (SP and ACT rings)
                for m in range(MT2):
                    for p in range(NPAIR):
                        nc.tensor.matmul(
                            oph[m][:], w2all[:, p, m, :], us[p][:],
                            start=(p == 0 and not general),
                            stop=(p == NPAIR - 1),
                        )
                    if m % 2 == 0:
                        nc.vector.tensor_copy(otb[:, m, :], oph[m][:])
                    else:
                        nc.scalar.copy(otb[:, m, :], oph[m][:])
                    if m == 1:
                        nc.sync.dma_start(
                            outT[:, 0:2, bh, :], otb[:, 0:2, :]
                        )
                    elif m == 3:
                        nc.scalar.dma_start(
                            outT[:, 2:4, bh, :], otb[:, 2:4, :]
                        )

    nc.compile()
    return nc


def make_in_maps(x, blending_coef, w1, b1, w2, b2, gamma, beta,
                 general: bool = False):
    import ml_dtypes

    f32 = np.float32
    bf = ml_dtypes.bfloat16
    x = np.asarray(x, f32)
    coef = np.asarray(blending_coef, f32)
    w1 = np.asarray(w1, f32)
    w2 = np.asarray(w2, f32)

    xT = np.ascontiguousarray(
        x.T.reshape(KT1, 128, B).transpose(1, 0, 2)
    )  # [128, KT1, B]
    coefT = np.ascontiguousarray(coef.T)  # [E, B]
    # cb4[j, p, b] = coef[b, 2p + (j >= 64)]
    cb4 = np.empty((128, NPAIR, B), f32)
    for p in range(NPAIR):
        cb4[:64, p, :] = coefT[2 * p]
        cb4[64:, p, :] = coefT[2 * p + 1]
    idd = np.ascontiguousarray(np.tile(np.eye(64, dtype=f32), (2, 2)))

    in_maps = []
    for c in range(N_CORES):
        sl = slice(c * HSL, (c + 1) * HSL)
        w1s = w1[:, :, sl]  # [E, IN, 64]
        w1L = np.ascontiguousarray(
            w1s.reshape(NPAIR, 2, KT1, 128, HSL)
            .transpose(3, 0, 2, 1, 4)  # [i, pair, k, eo, j]
            .reshape(128, NPAIR, KT1 * 128)
        )
        w2s = w2[:, sl, :]  # [E, 64, OUT]
        w2L = np.ascontiguousarray(
            w2s.reshape(NPAIR, 2, HSL, MT2, 128)
            .transpose(1, 2, 0, 3, 4)  # [eo, h, pair, m, j]
            .reshape(128, NPAIR, MT2 * 128)
        )
        m = {
            "xT": xT.astype(bf),
            "w1l": np.concatenate(
                [w1L.reshape(128, -1), idd], axis=1
            ).astype(bf),
            "w2l": w2L.reshape(128, -1).astype(bf),
            "cb4": cb4.astype(bf),
        }
        if general:
            b1a = np.asarray(b1, f32)
            b2a = np.asarray(b2, f32)
            cbbv = np.concatenate(
                [
                    coefT,
                    np.tile(b1a[:, sl], (1, 2)),
                    b2a if c == 0 else np.zeros_like(b2a),
                ],
                axis=1,
            )
            gbv = np.stack(
                [
                    np.tile(np.asarray(gamma, f32)[sl], 2),
                    np.tile(np.asarray(beta, f32)[sl], 2),
                ],
                axis=1,
            )
            m["cbb"] = np.ascontiguousarray(cbbv).astype(bf)
            m["gb"] = np.ascontiguousarray(gbv)
        in_maps.append(m)
    return in_maps


def combine_outputs(per_core_outs):
    """Sum the per-core partial outputs [128, MT2, NBH, 512] -> [B, OUT]."""
    acc = np.zeros((128, MT2, NBH, 512), np.float64)
    for o in per_core_outs:
        acc += np.asarray(o, np.float64)
    full = acc.transpose(1, 0, 2, 3).reshape(OUT, B)
    return np.ascontiguousarray(full.T.astype(np.float32))


def kernel(x, blending_coef, w1, b1, w2, b2, gamma, beta):
    global _CACHED_FAST, _CACHED_GEN
    from concourse.bass_utils import run_bass_kernel_spmd

    fast = (
        not np.any(np.asarray(b1))
        and not np.any(np.asarray(b2))
        and np.all(np.asarray(gamma) == 1.0)
        and not np.any(np.asarray(beta))
    )
    if fast:
        if _CACHED_FAST is None:
            _CACHED_FAST = build_nc(n_reps=1)
        nc = _CACHED_FAST
    else:
        if _CACHED_GEN is None:
            _CACHED_GEN = build_nc(n_reps=1, general=True)
        nc = _CACHED_GEN
    in_maps = make_in_maps(
        x, blending_coef, w1, b1, w2, b2, gamma, beta, general=not fast
    )
    res = run_bass_kernel_spmd(nc, in_maps, list(range(N_CORES)))
    return combine_outputs([res.results[c]["outT"] for c in range(N_CORES)])


# revision 30
# speedup vs baseline: 1.0018x; 1.0018x over previous
"""Bass/Trainium2 kernel for nn_ExpertMLP (soft-blended 8-expert MLP with
BatchNorm between the two layers).

Math (per sample b):
    h  = sum_e coef[b,e] * (x[b] @ w1[e])  + coef[b] @ b1
    hn = (h - mean_B(h)) * rsqrt(var_B(h) + eps) * gamma + beta
    h1 = elu(hn)
    out= sum_e coef[b,e] * (h1[b] @ w2[e]) + coef[b] @ b2

Sharding: HID (512) is split 64-per-core across 8 cores. Each core processes
the FULL batch for its HID slice, so the BatchNorm batch statistics are fully
local (no collective), and the per-expert weights are sharded (not
replicated). Layer 2 contracts only the local HID slice, so each core emits a
partial output [OUT, B]; the host sums the 8 partials and transposes.

Fast path (b1/b2 zero, gamma ones, beta zero — true for this problem's
setup_inputs): bf16 matmul inputs, host-precomputed coefficient broadcast
tiles, rsqrt via exp(-0.5*ln(var+eps)) so every ACT func (Ln/Exp/Relu) lives
in one activation-table set (no 1.3us table reloads), ELU's "-1" fused into
the per-pair coefficient multiplies, one consolidated DMA per tensor (each
HWDGE DMA occupies its ring ~2.3us on HW), and a staggered-reset hardware
loop (For_i(staggered_reset=True)) so consecutive iterations overlap instead
of paying a full all-engine drain at every back-edge. General path (nonzero
biases / affine) keeps the bias matmuls and BN affine.

On-chip layout is transposed (features on partitions, batch on the free dim):
  - L1: yp_pair[(e0|e1)*64hid, b] = sum_k W1L[p,k].T @ xT[k, b]
  - blend: multiply by per-expert coefficient tiles (host-broadcast), then a
    matmul with a tiled identity [I;I|I;I] sums the two expert halves of each
    pair, accumulating over the 4 pairs in PSUM, and duplicates h to both
    partition halves.
  - BN: bn_stats/bn_aggr over the free (batch) dim; rstd = exp(-0.5 ln(var+eps)).
  - ELU: t = min(exp(hn),1) + relu(hn); u_p = (t - 1) * coef_p  (fused).
  - L2: out[m] += W2L[p,m].T @ u_p  (m-major; each PSUM bank is copied to
    a packed SBUF tile and stored 2 m-tiles per DMA).
"""

import sys

sys.path.insert(0, "/opt/trn_rl_repo")

import numpy as np

E, IN, HID, OUT, B = 8, 512, 512, 512, 1024
BN_EPS = 1e-5
N_CORES = 8
HSL = HID // N_CORES  # 64: per-core hid slice
NPAIR = E // 2  # 4 expert pairs
KT1 = IN // 128  # 4 contraction tiles for layer 1
MT2 = OUT // 128  # 4 output row-tiles for layer 2
NBH = B // 512  # 2 batch halves (PSUM free-dim limit)
CBB_W = B + 128 + OUT  # packed coefT | b1dup | b2 widths (general path)

_CACHED_FAST = None
_CACHED_GEN = None


def build_nc(n_reps: int = 1, trace_sim: bool = False, loop_iters: int = 0,
             general: bool = False):
    from contextlib import ExitStack, nullcontext

    import concourse.bass as bass
    import concourse.tile as tile
    from concourse import bacc, mybir

    f32 = mybir.dt.float32
    bf16 = mybir.dt.bfloat16
    Alu = mybir.AluOpType
    Act = mybir.ActivationFunctionType

    nc = bacc.Bacc(
        "TRN2", target_bir_lowering=False, debug=False, num_devices=N_CORES
    )

    # one DMA per tensor: each HWDGE DMA occupies its ring ~2.3us on HW
    # (issue + transfer + completion receipt, unpipelined), so batch hard.
    xT = nc.dram_tensor("xT", [128, KT1, B], bf16, kind="ExternalInput")
    w1l = nc.dram_tensor("w1l", [128, NPAIR * KT1 * 128 + 128], bf16,
                         kind="ExternalInput")  # w1 tiles + idd appended
    w2l = nc.dram_tensor("w2l", [128, NPAIR * MT2 * 128], bf16, kind="ExternalInput")
    cb4 = nc.dram_tensor("cb4", [128, NPAIR, B], bf16, kind="ExternalInput")
    if general:
        cbb = nc.dram_tensor("cbb", [E, CBB_W], bf16, kind="ExternalInput")
        gb = nc.dram_tensor("gb", [128, 2], f32, kind="ExternalInput")
    outT = nc.dram_tensor("outT", [128, MT2, NBH, 512], bf16,
                          kind="ExternalOutput")

    with ExitStack() as ctx:
        tc = ctx.enter_context(tile.TileContext(nc, trace_sim=trace_sim))
        ins = ctx.enter_context(tc.tile_pool(name="ins", bufs=2))
        blend = ctx.enter_context(tc.tile_pool(name="blend", bufs=6))
        mids = ctx.enter_context(tc.tile_pool(name="mids", bufs=4))
        small = ctx.enter_context(tc.tile_pool(name="small", bufs=2))
        ups = ctx.enter_context(tc.tile_pool(name="ups", bufs=8))
        outs = ctx.enter_context(tc.tile_pool(name="outs", bufs=6))
        yps = ctx.enter_context(tc.tile_pool(name="yps", bufs=3, space="PSUM"))
        hps = ctx.enter_context(tc.tile_pool(name="hps", bufs=2, space="PSUM"))
        ops = ctx.enter_context(tc.tile_pool(name="ops", bufs=3, space="PSUM"))

        loop_cm = (tc.For_i(0, loop_iters, 1, staggered_reset=True,
                            hint_engines=tuple(mybir.ALL_ENGINES),
                            back_edge_label="kbk")
                   if loop_iters else nullcontext())
        with loop_cm:
          for _rep in range(n_reps):
            # ---- input loads.  SP: x + w1 + idd (earliest-needed);
            # gpsimd SWDGE: cb4 + w2 (needed a bit later); ACT kept free
            # for compute + the half-0 output stores.
            xall = ins.tile([128, KT1, B], bf16, tag="xts", name="xts", bufs=2)
            cbt = ins.tile([128, NPAIR, B], bf16, tag="cbt", name="cbt", bufs=2)
            if general:
                cbbt = ins.tile([E, CBB_W], bf16, tag="cbbt")
                nc.sync.dma_start(cbbt[:], cbb[:])
            # lead-in: SP carries x[k=0,1], gpsimd(SWDGE) x[k=2,3], ACT the
            # fused w1+idd block; cbt follows on ACT, w2 on SP (needed last)
            nc.sync.dma_start(xall[:, 0:2, :], xT[:, 0:2, :])
            w1i = ins.tile([128, NPAIR * KT1 * 128 + 128], bf16, tag="w1t",
                           name="w1t", bufs=2)
            nc.scalar.dma_start(w1i[:], w1l[:])
            nc.gpsimd.dma_start(xall[:, 2:4, :], xT[:, 2:4, :])
            iddt = w1i[:, NPAIR * KT1 * 128 : NPAIR * KT1 * 128 + 128]
            nc.scalar.dma_start(cbt[:], cb4[:])
            w2all = ins.tile([128, NPAIR, MT2, 128], bf16, tag="w2t",
                             name="w2t", bufs=2)
            w2v = w2all.rearrange("i p m j -> i (p m j)")
            nc.sync.dma_start(w2v[:], w2l[:])
            if general:
                gbt = small.tile([128, 2], f32, tag="gbt")
                nc.sync.dma_start(gbt[:], gb[:])
                ct = cbbt[:, 0:B]
                b1t = cbbt[:, B : B + 128]
                b2t = cbbt[:, B + 128 : CBB_W]
                gm = gbt[:, 0:1]
                bt = gbt[:, 1:2]

            # ---- layer 1 + blend + pair-sum ---------------------------------
            # All 8 yp matmul groups run back-to-back; each pair-sum
            # id-matmul is slotted in only after its blend (DVE) has had a
            # full matmul-group's time to finish, so the ~0.6us cross-engine
            # hop latency never stalls the PE.
            stats = small.tile([128, NBH, 6], f32, tag="stats")
            h_ps = [hps.tile([128, 512], f32, tag="hps", name="hp")
                    for _ in range(NBH)]
            if general:
                for bh in range(NBH):
                    nc.tensor.matmul(
                        h_ps[bh][:], b1t, ct[:, bh * 512 : bh * 512 + 512],
                        start=True, stop=False,
                    )
            bls = {}
            for bh in range(NBH):
                bsl = slice(bh * 512, (bh + 1) * 512)
                for p in range(NPAIR):
                    yp = yps.tile([128, 512], f32, tag="yps", name="yp")
                    for k in range(KT1):
                        nc.tensor.matmul(
                            yp[:],
                            w1i[:, (p * KT1 + k) * 128 : (p * KT1 + k + 1) * 128],
                            xall[:, k, bsl],
                            start=(k == 0), stop=(k == KT1 - 1),
                        )
                    bl = blend.tile([128, 512], bf16, tag="bl", name="bl")
                    nc.vector.tensor_mul(bl[:], yp[:], cbt[:, p, bsl])
                    bls[(bh, p)] = bl
                    if bh == 1:
                        # half-0's pair-sum rides between half-1's yp groups
                        nc.tensor.matmul(
                            h_ps[0][:], iddt, bls[(0, p)][:],
                            start=(p == 0 and not general),
                            stop=(p == NPAIR - 1),
                        )
            nc.vector.bn_stats(out=stats[:, 0, :], in_=h_ps[0][:])
            for p in range(NPAIR):
                nc.tensor.matmul(
                    h_ps[1][:], iddt, bls[(1, p)][:],
                    start=(p == 0 and not general), stop=(p == NPAIR - 1),
                )
            nc.vector.bn_stats(out=stats[:, 1, :], in_=h_ps[1][:])

            # ---- batch-norm: rstd = exp(-0.5 ln(var + eps)) -----------------
            # (Ln/Exp/Relu/Copy share one ACT table set -> no 1.3us reloads)
            eps = small.tile([128, 1], f32, tag="eps")
            nc.vector.memset(eps, BN_EPS)
            zzb = small.tile([128, 1], f32, tag="zzb")
            nc.vector.memset(zzb, 0.0)
            mv = small.tile([128, 2], f32, tag="mv")
            nc.vector.bn_aggr(out=mv[:], in_=stats[:])
            lv = small.tile([128, 1], f32, tag="lv")
            nc.scalar.activation(lv[:], mv[:, 1:2], Act.Ln, bias=eps[:])
            if general:
                rstd = small.tile([128, 1], f32, tag="rstd")
                nc.scalar.activation(rstd[:], lv[:], Act.Exp, bias=zzb[:], scale=-0.5)
                ns = small.tile([128, 1], f32, tag="ns")
                nc.vector.tensor_mul(ns[:], rstd[:], gm)
                nb0 = small.tile([128, 1], f32, tag="nb0")
                nc.vector.tensor_mul(nb0[:], mv[:, 0:1], ns[:])
                nb = small.tile([128, 1], f32, tag="nb")
                nc.vector.tensor_sub(nb[:], bt, nb0[:])
            else:
                ns = small.tile([128, 1], f32, tag="ns")
                nc.scalar.activation(ns[:], lv[:], Act.Exp, bias=zzb[:], scale=-0.5)
                nb = small.tile([128, 1], f32, tag="nb")
                nc.vector.scalar_tensor_tensor(
                    out=nb[:], in0=mv[:, 0:1], scalar=-1.0, in1=ns[:],
                    op0=Alu.mult, op1=Alu.mult,
                )
            if loop_iters:
                tc.mark_branch_hint_location("kbk", engines=mybir.ALL_ENGINES)
            # keep the PE activity window warm through the BN tail
            for _w in range(2):
                wps = ops.tile([128, 512], f32, tag="ops", name="wps")
                nc.tensor.matmul(
                    wps[:], iddt, xall[:, _w, 0:512], start=True, stop=True
                )

            # ---- ELU + blend + layer 2 --------------------------------------
            # hn is computed on DVE (the only engine that touches the hp PSUM
            # banks - Tile serializes cross-engine same-bank PSUM access, so
            # ACT reads hn from SBUF instead). t1 = elu(hn) pieces use
            # fast-mode tensor_scalar/tensor_tensor ops.
            zz = small.tile([128, 1], f32, tag="zz")
            nc.vector.memset(zz, 0.0)
            for bh in range(NBH):
                bsl = slice(bh * 512, (bh + 1) * 512)
                hn = mids.tile([128, 512], bf16, tag="hn", name="hn")
                nc.vector.tensor_scalar(
                    out=hn[:], in0=h_ps[bh][:], scalar1=ns[:], scalar2=nb[:],
                    op0=Alu.mult, op1=Alu.add,
                )
                rl = mids.tile([128, 512], bf16, tag="rl", name="rl")
                nc.vector.tensor_scalar(
                    out=rl[:], in0=hn[:], scalar1=0.0, scalar2=None,
                    op0=Alu.max,
                )
                expd = mids.tile([128, 512], bf16, tag="expd", name="expd")
                nc.scalar.activation(expd[:], hn[:], Act.Exp, bias=zz[:])
                a = mids.tile([128, 512], bf16, tag="a", name="a")
                nc.vector.tensor_scalar(
                    out=a[:], in0=expd[:], scalar1=1.0, scalar2=1.0,
                    op0=Alu.min, op1=Alu.subtract,
                )
                t1 = mids.tile([128, 512], bf16, tag="t1", name="t1")
                nc.vector.tensor_add(t1[:], a[:], rl[:])
                if bh == 0:
                    # mid-gap PE keep-warm riding on the ELU intermediates
                    for wi, wsrc in enumerate((hn, expd, a)):
                        wps2 = ops.tile([128, 512], f32, tag="ops",
                                        name="wps")
                        nc.tensor.matmul(
                            wps2[:], iddt, wsrc[:], start=True, stop=True
                        )
                us = []
                for p in range(NPAIR):
                    u = ups.tile([128, 512], bf16, tag="u", name="u")
                    nc.vector.tensor_mul(u[:], t1[:], cbt[:, p, bsl])
                    us.append(u)
                oph = [ops.tile([128, 512], f32, tag="ops", name="op")
                       for _m in range(MT2)]
                if general:
                    for m in range(MT2):
                        nc.tensor.matmul(
                            oph[m][:], b2t[:, m * 128 : (m + 1) * 128],
                            ct[:, bsl], start=True, stop=False,
                        )
                otb = outs.tile([128, MT2, 512], bf16, tag="otb", name="otb")
                # m-major: each op bank completes in turn; copy it into the
                # packed out tile, store 2 m-tiles per DMA (SP and ACT rings)
                for m in range(MT2):
                    for p in range(NPAIR):
                        nc.tensor.matmul(
                            oph[m][:], w2all[:, p, m, :], us[p][:],
                            start=(p == 0 and not general),
                            stop=(p == NPAIR - 1),
                        )
                    if m % 2 == 0:
                        nc.vector.tensor_copy(otb[:, m, :], oph[m][:])
                    else:
                        nc.scalar.copy(otb[:, m, :], oph[m][:])
                    if m == 1:
                        nc.sync.dma_start(
                            outT[:, 0:2, bh, :], otb[:, 0:2, :]
                        )
                    elif m == 3:
                        nc.scalar.dma_start(
                            outT[:, 2:4, bh, :], otb[:, 2:4, :]
                        )

    nc.compile()
    return nc


def make_in_maps(x, blending_coef, w1, b1, w2, b2, gamma, beta,
                 general: bool = False):
    import ml_dtypes

    f32 = np.float32
    bf = ml_dtypes.bfloat16
    x = np.asarray(x, f32)
    coef = np.asarray(blending_coef, f32)
    w1 = np.asarray(w1, f32)
    w2 = np.asarray(w2, f32)

    xT = np.ascontiguousarray(
        x.T.reshape(KT1, 128, B).transpose(1, 0, 2)
    )  # [128, KT1, B]
    coefT = np.ascontiguousarray(coef.T)  # [E, B]
    # cb4[j, p, b] = coef[b, 2p + (j >= 64)]
    cb4 = np.empty((128, NPAIR, B), f32)
    for p in range(NPAIR):
        cb4[:64, p, :] = coefT[2 * p]
        cb4[64:, p, :] = coefT[2 * p + 1]
    idd = np.ascontiguousarray(np.tile(np.eye(64, dtype=f32), (2, 2)))

    in_maps = []
    for c in range(N_CORES):
        sl = slice(c * HSL, (c + 1) * HSL)
        w1s = w1[:, :, sl]  # [E, IN, 64]
        w1L = np.ascontiguousarray(
            w1s.reshape(NPAIR, 2, KT1, 128, HSL)
            .transpose(3, 0, 2, 1, 4)  # [i, pair, k, eo, j]
            .reshape(128, NPAIR, KT1 * 128)
        )
        w2s = w2[:, sl, :]  # [E, 64, OUT]
        w2L = np.ascontiguousarray(
            w2s.reshape(NPAIR, 2, HSL, MT2, 128)
            .transpose(1, 2, 0, 3, 4)  # [eo, h, pair, m, j]
            .reshape(128, NPAIR, MT2 * 128)
        )
        m = {
            "xT": xT.astype(bf),
            "w1l": np.concatenate(
                [w1L.reshape(128, -1), idd], axis=1
            ).astype(bf),
            "w2l": w2L.reshape(128, -1).astype(bf),
            "cb4": cb4.astype(bf),
        }
        if general:
            b1a = np.asarray(b1, f32)
            b2a = np.asarray(b2, f32)
            cbbv = np.concatenate(
                [
                    coefT,
                    np.tile(b1a[:, sl], (1, 2)),
                    b2a if c == 0 else np.zeros_like(b2a),
                ],
                axis=1,
            )
            gbv = np.stack(
                [
                    np.tile(np.asarray(gamma, f32)[sl], 2),
                    np.tile(np.asarray(beta, f32)[sl], 2),
                ],
                axis=1,
            )
            m["cbb"] = np.ascontiguousarray(cbbv).astype(bf)
            m["gb"] = np.ascontiguousarray(gbv)
        in_maps.append(m)
    return in_maps


def combine_outputs(per_core_outs):
    """Sum the per-core partial outputs [128, MT2, NBH, 512] -> [B, OUT]."""
    acc = np.zeros((128, MT2, NBH, 512), np.float64)
    for o in per_core_outs:
        acc += np.asarray(o, np.float64)
    full = acc.transpose(1, 0, 2, 3).reshape(OUT, B)
    return np.ascontiguousarray(full.T.astype(np.float32))


def kernel(x, blending_coef, w1, b1, w2, b2, gamma, beta):
    global _CACHED_FAST, _CACHED_GEN
    from concourse.bass_utils import run_bass_kernel_spmd

    fast = (
        not np.any(np.asarray(b1))
        and not np.any(np.asarray(b2))
        and np.all(np.asarray(gamma) == 1.0)
        and not np.any(np.asarray(beta))
    )
    if fast:
        if _CACHED_FAST is None:
            _CACHED_FAST = build_nc(n_reps=1)
        nc = _CACHED_FAST
    else:
        if _CACHED_GEN is None:
            _CACHED_GEN = build_nc(n_reps=1, general=True)
        nc = _CACHED_GEN
    in_maps = make_in_maps(
        x, blending_coef, w1, b1, w2, b2, gamma, beta, general=not fast
    )
    res = run_bass_kernel_spmd(nc, in_maps, list(range(N_CORES)))
    return combine_outputs([res.results[c]["outT"] for c in range(N_CORES)])
